# revision 1
# baseline (speedup 1.0000x reference)
"""FFM layer kernel for 8 Trainium2 NeuronCores.

Math (reference): x[B,39] = 13 dense cols + 26 sparse index cols (ints 0..99
stored as f32).  inputs[B,2613] = [dense | one_hot(sparse)], then
  linear = inputs @ w.T + b
  field  = einsum('bn,nfk->bfk', inputs, v)        # [B,39,16]
  cross  = 0.5*sum_k((sum_f field)^2 - sum_f field^2)
  out    = sigmoid(linear + cross)

Strategy: data-parallel over batch, 2048 rows/core.  On each core the one-hot
matrix is built on-device (DVE is_equal against an offset ramp), transposed
feature-major so it can be the stationary matmul operand:
  psum[128b, 625] = sum_chunks ohT_chunk[128f,128b].T @ vperm_chunk[128f,625]
with 625 output cols = 624 field cols (k-major, f-minor) + 1 linear col.
Feature rows are [1s row (bias) | 13 dense | zero pad to 32 | 26*100
one-hot | zero tail], packed into 21 chunks of 128 so the PE contracts at
full K=128.  fp16 operands, fp32 PSUM accumulation.  Epilogue: strided
row-reduce for s[b,k], squared row-reduce for sum field^2, sigmoid on the
scalar engine.  Throwaway warmup matmuls release the HAM clock throttle
during the DMA head; host tensors are partition-major so DMAs move long
contiguous runs.
"""

import sys

sys.path.insert(0, "/opt/trn_rl_repo")

import numpy as np

import concourse.tile as tile
from concourse import bacc, mybir
from concourse.bass_utils import run_bass_kernel_spmd

N_CORES = 8
B_FULL = 16384
BC = B_FULL // N_CORES  # 2048 rows per core
P = 128
N_DENSE = 13
N_SPARSE = 26
SPARSE_DIM = 100
N_FIELD = 39
K_DIM = 16
NCHUNK = 21
RTOT = NCHUNK * P       # 2688 padded feature rows
# device feature rows: 0 = const-ones (bias), 1..13 = dense, 14..31 = zero
# pad (so the one-hot region starts 32-aligned for the compare ops), then
# 26*100 one-hot rows, then zero tail
SP0 = 32                # first one-hot row
NFEAT_END = SP0 + N_SPARSE * SPARSE_DIM  # 2632
COLS = N_FIELD * K_DIM + 1  # 625: 624 field cols + linear col
GB = 4                  # batch tiles per group (4 psum tiles = 8 banks)

F16 = mybir.dt.float16
F32 = mybir.dt.float32
I8 = mybir.dt.int8

_prog_cache = {}


def _build_program(bc):
    """One SPMD program for a batch slice of `bc` rows (all cores identical)."""
    nbt = bc // P
    ngroups = nbt // GB
    assert nbt % GB == 0
    gw = GB * P  # one-hot column width built per group

    nc = bacc.Bacc("TRN2", target_bir_lowering=False, debug=False)
    # idxrep/vperm are laid out partition-major on the host so every DMA
    # moves one long contiguous run per partition (short descriptor runs
    # throttle the DMA engines well below their 2KB+ peak efficiency)
    idx_d = nc.declare_dram_parameter(
        "idxrep", [ngroups, P, NCHUNK, gw], I8, isOutput=False)
    xdn_d = nc.declare_dram_parameter("xdn", [SP0, bc], F16, isOutput=False)
    vp_d = nc.declare_dram_parameter(
        "vperm", [P, NCHUNK, COLS], F16, isOutput=False)
    ramp_d = nc.declare_dram_parameter("ramp", [P, NCHUNK], F32, isOutput=False)
    # y kept [128, nbt] (partition-major) so the single output DMA writes
    # contiguous 64B runs instead of 2048 scattered 4B elements; the host
    # transposes the 8KB at the end
    y_d = nc.declare_dram_parameter("y", [P, nbt], F32, isOutput=True)

    # idx chunk sub-batches: one DMA each, alternating HWDGE queues so
    # descriptor generation overlaps; tiny first sub so chunk 0 lands early
    # and the first matmuls can start
    ISUB = [(0, 2), (2, 7), (7, 11), (11, 16), (16, NCHUNK)]
    ISUB_ENG = ("sync", "scalar", "sync", "scalar", "sync")
    VSUB = [(0, 2), (2, 9), (9, NCHUNK)]
    VSUB_ENG = ("scalar", "sync", "scalar")

    with tile.TileContext(nc) as tc:
        with (
            tc.tile_pool(name="pers", bufs=1) as pers,
            tc.tile_pool(name="idxp", bufs=2) as idxp,
            tc.tile_pool(name="psum", bufs=4, space="PSUM") as psum,
            tc.tile_pool(name="epi", bufs=3) as epi,
        ):
            oh_t = []
            for c in range(NCHUNK):
                oh_t.append(pers.tile([P, bc], F16, tag=f"oh{c}", name=f"oh{c}"))
            y_all = pers.tile([P, nbt], F32, tag="yall")
            vp_all = pers.tile([P, NCHUNK, COLS], F16, tag="vp")

            def load_idx(g):
                c0, c1 = g * gw, (g + 1) * gw
                subs = []
                for (lo, hi), ename in zip(ISUB, ISUB_ENG):
                    eng = getattr(nc, ename)
                    it = idxp.tile([P, hi - lo, gw], I8, tag=f"idx{lo}",
                                   name="idx", bufs=2)
                    eng.dma_start(it[:], idx_d[g, :, lo:hi, :])
                    subs.append((lo, it))
                return subs

            # ramp and the staged chunk-0 head rows first (tiny, and they
            # gate the first compare/copy), then group 0 idx loads
            ramp_t = pers.tile([P, NCHUNK], F32, tag="ramp")
            nc.sync.dma_start(ramp_t[:], ramp_d[:])

            def load_vp(lo, hi, ename):
                getattr(nc, ename).dma_start(
                    vp_all[:, lo:hi, :], vp_d[:, lo:hi, :])

            # vp chunks 0-1 first on scalar: they gate the first matmuls
            load_vp(*VSUB[0], VSUB_ENG[0])
            # rows 0..31 of chunk 0: row 0 = bias ones, 1..13 = dense x,
            # 14..31 = zeros (prebaked host-side); per group a single DVE
            # copy drops them into oh chunk 0.  The one-hot region starts at
            # row 32 so this never conflicts with the compares.
            xdn_t = pers.tile([SP0, bc], F16, tag="xdn")
            nc.scalar.dma_start(xdn_t[:], xdn_d[:])
            subs0 = load_idx(0)
            for (lo, hi), ename in list(zip(VSUB, VSUB_ENG))[1:]:
                load_vp(lo, hi, ename)

            # PE warmup: throwaway matmuls on zeroed tiles during the DMA
            # head release the HAM clock throttle (cold PE runs at 1.2GHz
            # until ~3.4us of sustained activity) so the real matmuls start
            # at 2.4GHz; the N=64 tail keeps the PE busy right up to when
            # the first one-hot chunk is ready without delaying it much
            wz16 = pers.tile([P, 16], F16, tag="wz16")
            wz512 = pers.tile([P, 512], F16, tag="wz512")
            nc.gpsimd.memset(wz16[:], 0.0)
            nc.gpsimd.memset(wz512[:], 0.0)
            wps = psum.tile([P, COLS], F32, tag="ps", name="warmps")
            for _ in range(10):
                nc.tensor.matmul(wps[0:16, 0:512], wz16[:], wz512[:],
                                 start=True, stop=True)
            for _ in range(40):
                nc.tensor.matmul(wps[0:16, 0:64], wz16[:], wz512[:, 0:64],
                                 start=True, stop=True)

            for g in range(ngroups):
                c0, c1 = g * gw, (g + 1) * gw
                subs = subs0 if g == 0 else load_idx(g)
                # one is_equal per chunk builds the one-hot columns; chunk 0
                # splits into [32:64)+[64:128) (start partitions must be
                # 32-aligned and 32-start allows at most 32 rows).  Group 0
                # builds in two column passes: a narrow bt0-only pass that
                # outruns PE consumption (~240ns vs 280ns per chunk), then
                # the rest while the PE works through bt0.
                passes = ((0, P), (P, gw)) if g == 0 else ((0, gw),)
                for pj0, pj1 in passes:
                    for si, (lo, it) in enumerate(subs):
                        for ci in range(it.shape[1]):
                            c = lo + ci
                            rngs = ((32, 64), (64, P)) if c == 0 else ((0, P),)
                            for r0, r1 in rngs:
                                nc.vector.tensor_scalar(
                                    out=oh_t[c][r0:r1, c0 + pj0:c0 + pj1],
                                    in0=it[r0:r1, ci, pj0:pj1],
                                    scalar1=ramp_t[r0:r1, c:c + 1],
                                    scalar2=None,
                                    op0=mybir.AluOpType.is_equal,
                                )
                        if si == 0:
                            # bias/dense/pad rows of chunk 0; issued after the
                            # first compares so a late xdn DMA can't
                            # head-of-line block the DVE queue
                            nc.vector.tensor_copy(
                                oh_t[0][0:SP0, c0 + pj0:c0 + pj1],
                                xdn_t[:, c0 + pj0:c0 + pj1])
                for b4 in range(GB):
                    bt = g * GB + b4
                    ps = psum.tile([P, COLS], F32, tag="ps")
                    for c in range(NCHUNK):
                        lhs = oh_t[c][:, bt * P:(bt + 1) * P]
                        nc.tensor.matmul(
                            ps[:, 0:512], lhs, vp_all[:, c, 0:512],
                            start=(c == 0), stop=(c == NCHUNK - 1),
                        )
                        nc.tensor.matmul(
                            ps[:, 512:COLS], lhs, vp_all[:, c, 512:COLS],
                            start=(c == 0), stop=(c == NCHUNK - 1),
                        )
                    # epilogue: s[b,k] = sum_f field, then cross + sigmoid
                    lin_t = epi.tile([P, 1], F32, tag="lin")
                    nc.vector.tensor_copy(lin_t[:], ps[:, COLS - 1:COLS])
                    s_t = epi.tile([P, K_DIM], F32, tag="s")
                    nc.vector.tensor_reduce(
                        out=s_t[:],
                        in_=ps[:, 0:COLS - 1].rearrange("p (k f) -> p k f", f=N_FIELD),
                        axis=mybir.AxisListType.X,
                        op=mybir.AluOpType.add,
                    )
                    sq_scr = epi.tile([P, COLS - 1], F32, tag="sqscr")
                    sqsum = epi.tile([P, 1], F32, tag="sqsum")
                    nc.scalar.activation(
                        out=sq_scr[:], in_=ps[:, 0:COLS - 1],
                        func=mybir.ActivationFunctionType.Square,
                        accum_out=sqsum[:],
                    )
                    # b2 = lin - 0.5*sqsum off the critical path: the final
                    # chain is then s_red -> square-accum -> sigmoid only
                    b2_t = epi.tile([P, 1], F32, tag="b2")
                    nc.vector.tensor_scalar(
                        out=b2_t[:], in0=sqsum[:],
                        scalar1=-0.5, scalar2=lin_t[:],
                        op0=mybir.AluOpType.mult,
                        op1=mybir.AluOpType.add,
                    )
                    s2_scr = epi.tile([P, K_DIM], F32, tag="s2scr")
                    s2sum = epi.tile([P, 1], F32, tag="s2sum")
                    nc.scalar.activation(
                        out=s2_scr[:], in_=s_t[:],
                        func=mybir.ActivationFunctionType.Square,
                        accum_out=s2sum[:],
                    )
                    nc.scalar.activation(
                        out=y_all[:, bt:bt + 1], in_=s2sum[:],
                        func=mybir.ActivationFunctionType.Sigmoid,
                        scale=0.5, bias=b2_t[:],
                    )
            nc.sync.dma_start(y_d[:], y_all[:])

    nc.compile()
    return nc


def _get_program(bc):
    if bc not in _prog_cache:
        _prog_cache[bc] = _build_program(bc)
    return _prog_cache[bc]


def _prep_shared(w_weight, w_bias, v):
    """vperm[RTOT, 625] fp16 and ramp[128, 21] f32 (same on every core)."""
    vperm = np.zeros((RTOT, COLS), np.float32)
    # cols j<624: j = k*39 + f  <->  v[n, f, k];  col 624 = linear weight
    v2 = np.ascontiguousarray(v.transpose(0, 2, 1)).reshape(2613, COLS - 1)
    vperm[1:1 + N_DENSE, :COLS - 1] = v2[:N_DENSE]
    vperm[1:1 + N_DENSE, COLS - 1] = w_weight[0, :N_DENSE]
    vperm[SP0:NFEAT_END, :COLS - 1] = v2[N_DENSE:]
    vperm[SP0:NFEAT_END, COLS - 1] = w_weight[0, N_DENSE:]
    vperm[0, COLS - 1] = float(w_bias[0])
    # partition-major [128, chunk, 625] so the DMA reads 26KB/partition runs
    vperm16 = np.ascontiguousarray(
        vperm.astype(np.float16).reshape(NCHUNK, P, COLS).transpose(1, 0, 2))

    r = np.arange(RTOT)
    in_sparse = (r >= SP0) & (r < NFEAT_END)
    off = np.where(in_sparse, (r - SP0) % SPARSE_DIM, 0)
    ramp = off.reshape(NCHUNK, P).T.astype(np.float32)
    ramp = np.ascontiguousarray(ramp)
    s_of_r = np.where(in_sparse, (r - SP0) // SPARSE_DIM, -1)
    return vperm16, ramp, s_of_r, in_sparse


def _prep_core(x_core, s_of_r, in_sparse):
    """Per-core idxrep[RTOT, bc] fp16 and dense xdn[13, bc] fp16."""
    bc = x_core.shape[0]
    idxrep = np.full((RTOT, bc), -1, np.int8)
    cols = (N_DENSE + s_of_r[in_sparse]).astype(np.int64)
    idxrep[in_sparse] = x_core[:, cols].T.astype(np.int8)
    # [group, p, chunk, gw] so each group sub-DMA reads contiguous
    # multi-KB runs per partition
    ngroups = bc // (GB * P)
    gw = GB * P
    idxrep = np.ascontiguousarray(
        idxrep.reshape(NCHUNK, P, ngroups, gw).transpose(2, 1, 0, 3))
    xdn = np.zeros((SP0, bc), np.float16)
    xdn[0] = 1.0
    xdn[1:1 + N_DENSE] = x_core[:, :N_DENSE].T.astype(np.float16)
    return idxrep, xdn


def run(x, w_weight, w_bias, v, trace=False, trace_kwargs=None):
    x = np.asarray(x, np.float32)
    w_weight = np.asarray(w_weight, np.float32)
    w_bias = np.asarray(w_bias, np.float32)
    v = np.asarray(v, np.float32)
    assert x.shape == (B_FULL, 39), x.shape

    vperm16, ramp, s_of_r, in_sparse = _prep_shared(w_weight, w_bias, v)
    in_maps = []
    for i in range(N_CORES):
        xc = x[i * BC:(i + 1) * BC]
        idxrep, xdn = _prep_core(xc, s_of_r, in_sparse)
        in_maps.append({
            "idxrep": idxrep,
            "xdn": xdn,
            "vperm": vperm16,
            "ramp": ramp,
        })

    nc = _get_program(BC)
    res = run_bass_kernel_spmd(
        nc, in_maps, list(range(N_CORES)),
        trace=trace, **(trace_kwargs or {}),
    )
    y = np.concatenate(
        [res.results[i]["y"].T.reshape(-1, 1) for i in range(N_CORES)], axis=0
    )
    return y.astype(np.float32), res


def kernel(x, w_weight, w_bias, v):
    y, _ = run(x, w_weight, w_bias, v)
    return y



# revision 4
# speedup vs baseline: 1.3986x; 1.3986x over previous
"""FFM layer kernel for 8 Trainium2 NeuronCores — fp8 DoubleRow edition.

Math (reference): x[B,39] = 13 dense cols + 26 sparse index cols (ints 0..99
stored as f32).  inputs[B,2613] = [dense | one_hot(sparse)], then
  linear = inputs @ w.T + b
  field  = einsum('bn,nfk->bfk', inputs, v)        # [B,39,16]
  cross  = 0.5*sum_k((sum_f field)^2 - sum_f field^2)
  out    = sigmoid(linear + cross)

Strategy: data-parallel over batch, 2048 rows/core.  The sparse part is an
embedding-bag done as a one-hot matmul; the one-hot lhs is exact in fp8, so
the whole sparse contraction runs as fp8e4 MatmulPerfMode.DoubleRow (two
128-row k-chunks per instruction, 2x PE throughput).  PSUM columns (658):
  [0:624)   field, k-major f-minor, v quantized fp8 at scale 1024 (sq only)
  [624:641) hi block: 16 U_hi cols (U = sum_f v, scale 512) + w_hi col
            (scale 1024, bias folded into the dense matmul)
  [641:658) lo block: fp8 quantization residuals at 16x the hi scale
s and linear are reconstructed as hi + lo/16, which recovers ~fp16 accuracy
for the dominant (sum_f field)^2 term; the field cols feed only sum field^2
where fp8 error is quadratically suppressed.  The 13 dense features + bias
row contract in a separate fp16 matmul into the same PSUM group.  One-hot is
built on-device (DVE is_equal against an offset ramp) into 11 pair tiles
[128, 2, bc]; host tensors are partition-major so DMAs move long contiguous
runs; throwaway warmup matmuls release the HAM clock throttle during the
DMA head.
"""

import sys

sys.path.insert(0, "/opt/trn_rl_repo")

import numpy as np
import ml_dtypes

import concourse.tile as tile
from concourse import bacc, mybir
from concourse.bass_utils import run_bass_kernel_spmd

N_CORES = 8
B_FULL = 16384
BC = B_FULL // N_CORES  # 2048 rows per core
P = 128
N_DENSE = 13
N_SPARSE = 26
SPARSE_DIM = 100
N_FIELD = 39
K_DIM = 16
NCHUNK = 21             # sparse one-hot chunks of 128 rows (2600 real rows)
NPAIR = 11              # DoubleRow pairs (pair 10's second chunk is zeros)
NSPROWS = N_SPARSE * SPARSE_DIM  # 2600
COLS = 658              # 624 field | 16 U_hi + w_hi | 16 U_lo + w_lo
FLD = N_FIELD * K_DIM   # 624
HI0 = FLD               # 624
LO0 = FLD + K_DIM + 1   # 641
GB = 4                  # batch tiles per group (4 psum tiles = 8 banks)

SC_V = 1024.0
SC_U = 512.0
SC_W = 1024.0
LOGAIN = 16.0           # lo cols stored at hi_scale * LOGAIN

F16 = mybir.dt.float16
F32 = mybir.dt.float32
F8 = mybir.dt.float8e4
I8 = mybir.dt.int8

_prog_cache = {}


def _build_program(bc):
    """One SPMD program for a batch slice of `bc` rows (all cores identical)."""
    nbt = bc // P
    ngroups = nbt // GB
    assert nbt % GB == 0
    gw = GB * P  # one-hot column width built per group

    nc = bacc.Bacc("TRN2", target_bir_lowering=False, debug=False)
    # all dram tensors are partition-major on the host so every DMA moves one
    # long contiguous run per partition
    idx_d = nc.declare_dram_parameter(
        "idxrep", [ngroups, P, NCHUNK, gw], I8, isOutput=False)
    xdn_d = nc.declare_dram_parameter("xdn", [1 + N_DENSE, bc], F16,
                                      isOutput=False)
    vd_d = nc.declare_dram_parameter("vdense", [1 + N_DENSE, COLS], F16,
                                     isOutput=False)
    vp_d = nc.declare_dram_parameter(
        "vperm", [P, NPAIR, 2, COLS], F8, isOutput=False)
    ramp_d = nc.declare_dram_parameter("ramp", [P, NCHUNK], F32, isOutput=False)
    y_d = nc.declare_dram_parameter("y", [P, nbt], F32, isOutput=True)

    # idx chunk sub-batches: one DMA each, alternating HWDGE queues so
    # descriptor generation overlaps; tiny first sub so chunk 0 lands early
    ISUB = [(0, 2), (2, 7), (7, 11), (11, 16), (16, NCHUNK)]
    ISUB_ENG = ("sync", "scalar", "sync", "scalar", "sync")

    with tile.TileContext(nc) as tc:
        with (
            tc.tile_pool(name="pers", bufs=1) as pers,
            tc.tile_pool(name="idxp", bufs=2) as idxp,
            tc.tile_pool(name="psum", bufs=4, space="PSUM") as psum,
            tc.tile_pool(name="epi", bufs=3) as epi,
        ):
            oh_t = []
            for p_ in range(NPAIR):
                oh_t.append(pers.tile([P, 2, bc], F8, tag=f"oh{p_}",
                                      name=f"oh{p_}"))
            y_all = pers.tile([P, nbt], F32, tag="yall")
            vp_all = pers.tile([P, NPAIR, 2, COLS], F8, tag="vp")
            vd_t = pers.tile([1 + N_DENSE, COLS], F16, tag="vd")
            xdn_t = pers.tile([1 + N_DENSE, bc], F16, tag="xdn")
            ramp_t = pers.tile([P, NCHUNK], F32, tag="ramp")

            def load_idx(g):
                subs = []
                for (lo, hi), ename in zip(ISUB, ISUB_ENG):
                    eng = getattr(nc, ename)
                    it = idxp.tile([P, hi - lo, gw], I8, tag=f"idx{lo}",
                                   name="idx", bufs=2)
                    eng.dma_start(it[:], idx_d[g, :, lo:hi, :])
                    subs.append((lo, it))
                return subs

            # small gating tensors first, then group-0 idx, then the big vp
            nc.sync.dma_start(ramp_t[:], ramp_d[:])
            nc.scalar.dma_start(vd_t[:], vd_d[:])
            nc.scalar.dma_start(xdn_t[:], xdn_d[:])
            nc.scalar.dma_start(vp_all[:, 0:3, :, :], vp_d[:, 0:3, :, :])
            subs0 = load_idx(0)
            nc.scalar.dma_start(vp_all[:, 3:NPAIR, :, :],
                                vp_d[:, 3:NPAIR, :, :])

            # pair 10's second chunk is never written by the one-hot build;
            # zero it once so uninitialized SBUF can't poison the matmul
            nc.vector.memset(oh_t[NPAIR - 1][:, 1, :], 0.0)

            # PE warmup: throwaway matmuls on zeroed tiles during the DMA
            # head release the HAM clock throttle (cold PE runs at 1.2GHz
            # until ~3.4us of sustained activity) so the real matmuls start
            # at 2.4GHz
            wz8 = pers.tile([P, 2, 16], F8, tag="wz8")
            wz8b = pers.tile([P, 2, 256], F8, tag="wz8b")
            nc.gpsimd.memset(wz8[:], 0.0)
            nc.gpsimd.memset(wz8b[:], 0.0)
            wps = psum.tile([P, COLS], F32, tag="ps", name="warmps")
            for _ in range(10):
                nc.tensor.matmul(wps[0:16, 0:256], wz8[:], wz8b[:],
                                 start=True, stop=True,
                                 perf_mode=mybir.MatmulPerfMode.DoubleRow)
            for _ in range(60):
                nc.tensor.matmul(wps[0:16, 0:64], wz8[:], wz8b[:, :, 0:64],
                                 start=True, stop=True,
                                 perf_mode=mybir.MatmulPerfMode.DoubleRow)

            for g in range(ngroups):
                c0, c1 = g * gw, (g + 1) * gw
                subs = subs0 if g == 0 else load_idx(g)
                # one is_equal per chunk builds the one-hot columns.  Group 0
                # builds in two column passes: a narrow bt0-only pass so the
                # first matmuls can start early, then the rest.
                passes = ((0, P), (P, gw)) if g == 0 else ((0, gw),)
                for pj0, pj1 in passes:
                    for lo, it in subs:
                        for ci in range(it.shape[1]):
                            c = lo + ci
                            nc.vector.tensor_scalar(
                                out=oh_t[c // 2][:, c % 2,
                                                 c0 + pj0:c0 + pj1],
                                in0=it[:, ci, pj0:pj1],
                                scalar1=ramp_t[:, c:c + 1],
                                scalar2=None,
                                op0=mybir.AluOpType.is_equal,
                            )
                for b4 in range(GB):
                    bt = g * GB + b4
                    bs = slice(bt * P, (bt + 1) * P)
                    ps = psum.tile([P, COLS], F32, tag="ps")
                    # dense + bias rows, fp16
                    nc.tensor.matmul(ps[:, 0:512], xdn_t[:, bs],
                                     vd_t[:, 0:512], start=True, stop=False)
                    nc.tensor.matmul(ps[:, 512:COLS], xdn_t[:, bs],
                                     vd_t[:, 512:COLS], start=True,
                                     stop=False)
                    # sparse one-hot contraction, fp8 DoubleRow
                    for p_ in range(NPAIR):
                        lhs = oh_t[p_][:, :, bs]
                        last = p_ == NPAIR - 1
                        nc.tensor.matmul(
                            ps[:, 0:512], lhs, vp_all[:, p_, :, 0:512],
                            start=False, stop=last,
                            perf_mode=mybir.MatmulPerfMode.DoubleRow,
                        )
                        nc.tensor.matmul(
                            ps[:, 512:COLS], lhs, vp_all[:, p_, :, 512:COLS],
                            start=False, stop=last,
                            perf_mode=mybir.MatmulPerfMode.DoubleRow,
                        )
                    # epilogue: s/lin = hi + lo/16; sq = sum field^2;
                    # cross + sigmoid with all scale factors folded in
                    t1 = epi.tile([P, K_DIM + 1], F32, tag="t1")
                    nc.vector.tensor_scalar(
                        out=t1[:], in0=ps[:, LO0:COLS],
                        scalar1=1.0 / LOGAIN, scalar2=None,
                        op0=mybir.AluOpType.mult,
                    )
                    slin = epi.tile([P, K_DIM + 1], F32, tag="slin")
                    nc.vector.tensor_tensor(
                        out=slin[:], in0=t1[:], in1=ps[:, HI0:LO0],
                        op=mybir.AluOpType.add,
                    )
                    sq_scr = epi.tile([P, FLD], F32, tag="sqscr")
                    sqsum = epi.tile([P, 1], F32, tag="sqsum")
                    nc.scalar.activation(
                        out=sq_scr[:], in_=ps[:, 0:FLD],
                        func=mybir.ActivationFunctionType.Square,
                        accum_out=sqsum[:],
                    )
                    s2_scr = epi.tile([P, K_DIM], F32, tag="s2scr")
                    s2sum = epi.tile([P, 1], F32, tag="s2sum")
                    nc.vector.scalar_tensor_tensor(
                        out=s2_scr[:], in0=slin[:, 0:K_DIM],
                        scalar=1.0, in1=slin[:, 0:K_DIM],
                        op0=mybir.AluOpType.mult,
                        op1=mybir.AluOpType.mult,
                        accum_out=s2sum[:],
                    )
                    # b2 = lin - 0.5*sq (true units) off the critical path
                    b2a = epi.tile([P, 1], F32, tag="b2a")
                    nc.vector.tensor_scalar(
                        out=b2a[:], in0=slin[:, K_DIM:K_DIM + 1],
                        scalar1=1.0 / SC_W, scalar2=None,
                        op0=mybir.AluOpType.mult,
                    )
                    b2 = epi.tile([P, 1], F32, tag="b2")
                    nc.vector.tensor_scalar(
                        out=b2[:], in0=sqsum[:],
                        scalar1=-0.5 / (SC_V * SC_V), scalar2=b2a[:],
                        op0=mybir.AluOpType.mult,
                        op1=mybir.AluOpType.add,
                    )
                    nc.scalar.activation(
                        out=y_all[:, bt:bt + 1], in_=s2sum[:],
                        func=mybir.ActivationFunctionType.Sigmoid,
                        scale=0.5 / (SC_U * SC_U), bias=b2[:],
                    )
            nc.sync.dma_start(y_d[:], y_all[:])

    nc.compile()
    return nc


def _get_program(bc):
    if bc not in _prog_cache:
        _prog_cache[bc] = _build_program(bc)
    return _prog_cache[bc]


def _q8(a, scale):
    return np.clip(a * scale, -240.0, 240.0).astype(ml_dtypes.float8_e4m3)


def _prep_shared(w_weight, w_bias, v):
    """vperm fp8 [128, 11, 2, 658], vdense fp16 [14, 658], ramp [128, 21]."""
    # field col j = k*39 + f  <->  v[n, f, k]
    v2 = np.ascontiguousarray(v.transpose(0, 2, 1)).reshape(2613, FLD)
    u = v.sum(axis=1)                       # [2613, 16]
    w = w_weight[0]                         # [2613]

    # sparse rows -> fp8 hi (+ lo residual for U and w)
    rows = np.zeros((NPAIR * 2 * P, COLS), ml_dtypes.float8_e4m3)
    vs8 = _q8(v2[N_DENSE:], SC_V)
    uhi = _q8(u[N_DENSE:], SC_U)
    ulo = _q8(u[N_DENSE:] - uhi.astype(np.float32) / SC_U, SC_U * LOGAIN)
    whi = _q8(w[N_DENSE:], SC_W)
    wlo = _q8(w[N_DENSE:] - whi.astype(np.float32) / SC_W, SC_W * LOGAIN)
    rows[:NSPROWS, 0:FLD] = vs8
    rows[:NSPROWS, HI0:HI0 + K_DIM] = uhi
    rows[:NSPROWS, HI0 + K_DIM] = whi
    rows[:NSPROWS, LO0:LO0 + K_DIM] = ulo
    rows[:NSPROWS, LO0 + K_DIM] = wlo
    # device layout [part, pair, j, col]; sparse row = (pair*2+j)*128 + part
    vperm = np.ascontiguousarray(
        rows.reshape(NPAIR, 2, P, COLS).transpose(2, 0, 1, 3))

    # dense rows: row 0 = bias carrier (ones in xdn), rows 1..13 = x_dense
    vdense = np.zeros((1 + N_DENSE, COLS), np.float32)
    vdense[1:, 0:FLD] = v2[:N_DENSE] * SC_V
    vdense[1:, HI0:HI0 + K_DIM] = u[:N_DENSE] * SC_U
    vdense[1:, HI0 + K_DIM] = w[:N_DENSE] * SC_W
    vdense[0, HI0 + K_DIM] = float(w_bias[0]) * SC_W
    vdense16 = vdense.astype(np.float16)

    r = np.arange(NCHUNK * P)
    ramp = np.where(r < NSPROWS, r % SPARSE_DIM, -1).astype(np.float32)
    ramp = np.ascontiguousarray(ramp.reshape(NCHUNK, P).T)
    s_of_r = np.where(r < NSPROWS, r // SPARSE_DIM, -1)
    return vperm, vdense16, ramp, s_of_r


def _prep_core(x_core, s_of_r):
    """Per-core idxrep[ngroups, 128, 21, gw] int8 and xdn[14, bc] fp16."""
    bc = x_core.shape[0]
    rtot = NCHUNK * P
    idxrep = np.full((rtot, bc), -2, np.int8)
    valid = s_of_r >= 0
    cols = (N_DENSE + s_of_r[valid]).astype(np.int64)
    idxrep[valid] = x_core[:, cols].T.astype(np.int8)
    ngroups = bc // (GB * P)
    gw = GB * P
    idxrep = np.ascontiguousarray(
        idxrep.reshape(NCHUNK, P, ngroups, gw).transpose(2, 1, 0, 3))
    xdn = np.zeros((1 + N_DENSE, bc), np.float16)
    xdn[0] = 1.0
    xdn[1:] = x_core[:, :N_DENSE].T.astype(np.float16)
    return idxrep, xdn


def run(x, w_weight, w_bias, v, trace=False, trace_kwargs=None):
    x = np.asarray(x, np.float32)
    w_weight = np.asarray(w_weight, np.float32)
    w_bias = np.asarray(w_bias, np.float32)
    v = np.asarray(v, np.float32)
    assert x.shape == (B_FULL, 39), x.shape

    vperm, vdense16, ramp, s_of_r = _prep_shared(w_weight, w_bias, v)
    in_maps = []
    for i in range(N_CORES):
        xc = x[i * BC:(i + 1) * BC]
        idxrep, xdn = _prep_core(xc, s_of_r)
        in_maps.append({
            "idxrep": idxrep,
            "xdn": xdn,
            "vdense": vdense16,
            "vperm": vperm,
            "ramp": ramp,
        })

    nc = _get_program(BC)
    res = run_bass_kernel_spmd(
        nc, in_maps, list(range(N_CORES)),
        trace=trace, **(trace_kwargs or {}),
    )
    y = np.concatenate(
        [res.results[i]["y"].T.reshape(-1, 1) for i in range(N_CORES)], axis=0
    )
    return y.astype(np.float32), res


def kernel(x, w_weight, w_bias, v):
    y, _ = run(x, w_weight, w_bias, v)
    return y
